# revision 48
# baseline (speedup 1.0000x reference)
"""Causal GQA attention (S=2048, Hq=32, Hkv=8, D=128, fp32 IO) on 8 Trainium2
NeuronCores, sharded over heads: core i handles q-heads 4i..4i+3 and kv-head i
(no cross-core communication).

Per-core Bass/Tile kernel design (v2 — fp8 + dual-engine exp):
- Scores are computed TRANSPOSED (S^T[k, q], d on the contraction partitions)
  from host-pre-transposed fp16 Q/K, exactly as the fp16 baseline.
- The exp is split across BOTH the scalar engine (native Exp activation with a
  folded -1 bias, fp8e4 output) and the vector engine (Schraudolph fast-exp:
  one tensor_scalar computing round(score*A + B) into a uint8 tile whose bytes
  ARE the fp8e4 encoding of 2^((u-56)/8) ~= exp(score*SCALE - 1); the fp32->u8
  conversion saturates at 0/255 and rounds-to-nearest, HW-verified). Causal
  triangle masking for DVE-handled diagonal tiles is folded into the same op
  via scalar_tensor_tensor with a precomputed additive mask (-1e9 above the
  diagonal -> u8 0 -> +0.0 in fp8).
- The P*V accumulation runs in fp8 with MatmulPerfMode.DoubleRow: each matmul
  contracts TWO 128-row k-tiles (stationary = paired P^T tiles, moving =
  paired V tiles extended with a ones column that yields the softmax
  denominator for free in column 128).
- Early queries (chunk 0, q < 512) have too few softmax terms to average away
  fp8 noise, so chunk 0 runs the original fp16 path (fp16 pt, fp16 V, scalar
  exp, -30000 identity-matmul triangle masking).
- The AV PSUM quad [128, 4, 512] is copied to SBUF fp16 UNNORMALIZED (with the
  denominator riding in column 128) and DMA'd out; the host does the divide.
"""

from collections import deque
from contextlib import ExitStack

import numpy as np

import concourse.bass as bass
import concourse.mybir as mybir
import concourse.tile as tile
from concourse.mybir import ActivationFunctionType as AF
from concourse.alu_op_type import AluOpType
from concourse.vector_clock import ScopedClock
from concourse.bass_utils import run_bass_kernel_spmd

# Walrus's BIR-simulation pass is ~85% of NEFF compile time and is a
# verification-only pass; skip it.
try:
    import concourse.bass_utils as _bu

    if not getattr(_bu, "_birsim_patched", False):
        _orig_run_command = _bu.run_command

        def _fast_run_command(cmd, *a, **kw):
            cmd = [
                c.replace("--enable-birsim=true", "--enable-birsim=false")
                if isinstance(c, str)
                else c
                for c in cmd
            ]
            return _orig_run_command(cmd, *a, **kw)

        _bu.run_command = _fast_run_command
        _bu._birsim_patched = True
except Exception:
    pass

S = 2048
D = 128
P = 128
NT = S // P          # 16 k-tiles
CHUNK = 512          # q columns per score chunk
NCH = S // CHUNK     # 4 chunks
TPC = CHUNK // P     # 4 k-tiles / diag rows per chunk
VW = 132             # v_ext free width (128 d + 1 ones + 3 pad)
G = 2                # k-tiles per PSUM score group (2 banks; av quad takes 4)
SCALE = 0.08838834764831845
NEG = -30000.0
HL = 4               # q-heads per core
N_CORES = 8

LN2 = 0.6931471805599453
C_SHIFT = 1.0                      # exp(s*SCALE - C): keeps fp8 range safe
A_U8 = 8.0 * SCALE / LN2           # Schraudolph multiplier
B_U8 = 56.0 - 0.52 - 8.0 * C_SHIFT / LN2   # bias incl. -0.52 centering
MASK_NEG = -60000.0                # fp16-representable; a*s+MASK_NEG -> u8 0

F16 = mybir.dt.float16
F32 = mybir.dt.float32
F8 = mybir.dt.float8e4
U8 = mybir.dt.uint8
NPF8 = mybir.dt.np(F8)

WAIT_LIMIT = 1  # this image's walrus encodes at most one sync-wait per inst

# Engine-balance cost model (ns); static greedy assignment. HW-calibrated:
# scalar ACTIVATE = n*0.833 + 293; DVE tensor_scalar/stt = n*1.042 + 190
# (independent of output dtype / bias).
ACT_FIX, ACT_PER = 293.0, 0.833
DVE_FIX, DVE_PER = 190.0, 1.042


class SplitDrainTileContext(tile.TileContext):
    """TileContext whose exit drain spreads its semaphore waits over
    multiple SP instructions (walrus here caps sync-waits per inst)."""

    def _drain_and_barrier(self, tick_clock, wait_clock):
        drain_inst = self.nc.sync.drain()
        wait_clock.add_sem_waits(
            drain_inst.ins, ScopedClock({None: tick_clock.global_clock})
        )
        waits = list(drain_inst.ins.sync_info.on_wait)
        if len(waits) > WAIT_LIMIT:
            drain_inst.ins.sync_info = mybir.SyncInfo(
                on_wait=waits[:WAIT_LIMIT],
                on_update=list(drain_inst.ins.sync_info.on_update),
            )
            for i in range(WAIT_LIMIT, len(waits), WAIT_LIMIT):
                nop = self.nc.sync.nop(nofuse=True)
                nop.ins.sync_info = mybir.SyncInfo(
                    on_wait=waits[i : i + WAIT_LIMIT], on_update=[]
                )
        self.nc.all_engine_barrier()
        popped = self.nc._tile_sem_poison_stack.pop()
        assert popped is self._sem_poison
        self.nc.clear_and_free_semaphores(list(self.sems.allocated().values()))
        self.nc.all_engine_barrier()


def split_multi_waits(nc, limit: int = WAIT_LIMIT):
    """Spread >limit sync-waits onto same-engine NOPs inserted before the
    instruction (engines execute in order: cumulative semantics identical)."""
    n_split = 0
    for fn in nc.m.functions:
        for bb in fn.blocks:
            out = []
            changed = False
            for inst in bb.instructions:
                si = inst.sync_info
                waits = list(si.on_wait) if si is not None else []
                if len(waits) > limit:
                    changed = True
                    n_split += 1
                    extra = waits[:-limit]
                    for ci in range(0, len(extra), limit):
                        nop = mybir.InstNoOp(
                            name=f"{inst.name}-sw{ci}", ins=[], outs=[]
                        )
                        nop.engine = inst.engine
                        nop.sync_info = mybir.SyncInfo(
                            on_wait=extra[ci : ci + limit], on_update=[]
                        )
                        nc.register_instruction(nop, overwrite=True)
                        out.append(nop)
                    inst.sync_info = mybir.SyncInfo(
                        on_wait=waits[-limit:], on_update=list(si.on_update)
                    )
                out.append(inst)
            if changed:
                bb.instructions = out
    return n_split


def build_nc() -> bass.Bass:
    nc = bass.Bass()

    qT = nc.dram_tensor("qT", [HL, P, S], F16, kind="ExternalInput")
    kT = nc.dram_tensor("kT", [P, S], F16, kind="ExternalInput")
    vx8 = nc.dram_tensor("vx8", [S, VW], F8, kind="ExternalInput")
    vx16 = nc.dram_tensor("vx16", [CHUNK, VW], F16, kind="ExternalInput")
    mask = nc.dram_tensor("mask", [P, P], F16, kind="ExternalInput")
    ident = nc.dram_tensor("ident", [P, P], F16, kind="ExternalInput")
    # mask2[p, u] = B_U8 if u >= p else MASK_NEG; diag tile r at window
    # [off:] uses mask2[:, :CHUNK-off] (value depends only on j-off-p).
    mask2 = nc.dram_tensor("mask2", [P, CHUNK], F16, kind="ExternalInput")
    ox = nc.dram_tensor("ox", [HL, S, VW], F16, kind="ExternalOutput")

    # running per-engine cost estimates for static load balancing
    est = {"s": 0.0, "v": 0.0}

    def pick(scalar_cost, dve_cost):
        if est["s"] + scalar_cost <= est["v"] + dve_cost:
            est["s"] += scalar_cost
            return "s"
        est["v"] += dve_cost
        return "v"

    with SplitDrainTileContext(nc) as tc, ExitStack() as ctx:
        const = ctx.enter_context(tc.tile_pool(name="const", bufs=1))
        qpool = ctx.enter_context(tc.tile_pool(name="qpool", bufs=HL))
        pt8pool = ctx.enter_context(tc.tile_pool(name="pt8pool", bufs=2))
        pt16pool = ctx.enter_context(tc.tile_pool(name="pt16pool", bufs=2))
        opool = ctx.enter_context(tc.tile_pool(name="opool", bufs=2))
        psum_sc = ctx.enter_context(tc.tile_pool(name="psc", bufs=3, space="PSUM"))
        psum_av = ctx.enter_context(tc.tile_pool(name="pav", bufs=1, space="PSUM"))

        # Head 0 runs chunks ASCENDING, so the kernel can start computing
        # (h0, c0) after only kT[:, :512] + qT0[:, :512] + the tiny c0-path
        # consts have landed. Input DMAs are split across the sync and gpsimd
        # queues in 512-col pieces ordered by first use.
        kT_sb = const.tile([P, S], F16)
        qT_sbs = []
        qT_sb0 = qpool.tile([P, S], F16, tag="q")
        qT_sbs.append(qT_sb0)
        m_sb = const.tile([P, P], F16)
        i_sb = const.tile([P, P], F16)
        v16_sb = const.tile([P, TPC, VW], F16)
        v8_sb = const.tile([P, NT, VW], F8)
        mask2_sb = const.tile([P, CHUNK], F16)
        nbias = const.tile([P, 1], F32)

        # gpsimd issues NO DMAs (its DGE drain at kernel exit costs ~3.5us
        # once used). Early small pieces ride the scalar queue before its
        # first exp can run; the late bulk (v8, qT0 tail, qT2) is emitted
        # MID-PROGRAM on the scalar queue (see the main loop) where the
        # engine has slack, with est[] charged for the issue time.
        nc.sync.dma_start(kT_sb[:, :CHUNK], kT[:, :CHUNK])
        nc.gpsimd.dma_start(qT_sb0[:, :CHUNK], qT[0, :, :CHUNK])
        nc.sync.dma_start(m_sb[:], mask[:])
        nc.sync.dma_start(i_sb[:], ident[:])
        nc.vector.memset(nbias[:], -C_SHIFT)
        nc.gpsimd.dma_start(
            qT_sb0[:, CHUNK : 2 * CHUNK], qT[0, :, CHUNK : 2 * CHUNK]
        )
        nc.sync.dma_start(kT_sb[:, CHUNK : 2 * CHUNK], kT[:, CHUNK : 2 * CHUNK])
        nc.gpsimd.dma_start(mask2_sb[:], mask2[:])
        nc.sync.dma_start(v16_sb[:], vx16.rearrange("(t p) w -> p t w", p=P))
        nc.sync.dma_start(kT_sb[:, 2 * CHUNK :], kT[:, 2 * CHUNK :])
        for h in range(1, HL):
            qT_sb = qpool.tile([P, S], F16, tag="q")
            if h != 2:
                nc.sync.dma_start(qT_sb[:], qT[h])
            qT_sbs.append(qT_sb)

        def late_dmas(step):
            """Mid-program input DMAs on the scalar queue — ONE per chunk so
            consecutive transfers never serialize the queue ahead of exps
            (each dma issue waits the previous transfer on the DGE ring)."""
            if step == 0:
                est["s"] += 1100.0
                nc.scalar.dma_start(
                    v8_sb[:], vx8.rearrange("(t p) w -> p t w", p=P)
                )
            elif step == 1:
                est["s"] += 900.0
                nc.scalar.dma_start(
                    qT_sb0[:, 2 * CHUNK :], qT[0, :, 2 * CHUNK :]
                )
            elif step == 2:
                est["s"] += 1700.0
                nc.scalar.dma_start(qT_sbs[2][:], qT[2])

        def emit_qk_exp(h, c, work):
            """QK + exp for one (head, chunk). After each PSUM score group is
            emitted, one pending AV thunk from the previous chunk is emitted
            so the PE interleaves AV matmuls with QK instead of stalling on
            the exp engines draining score PSUM (stalls also drop the PE out
            of its fast p-state).

            c == 0: fp16 path (trimmed QK + identity-matmul triangle mask,
            scalar exp -> fp16 pt). c >= 1: fp8 path (no mask matmuls;
            full-tile groups exp'd on scalar Exp->fp8 or DVE Schraudolph->u8;
            diagonal tiles on DVE scalar_tensor_tensor with the mask4
            additive table)."""
            qT_sb = qT_sbs[h]
            ntiles = TPC * (c + 1)
            if c == 0:
                pt = pt16pool.tile([P, TPC, CHUNK], F16, tag="pt16")
            else:
                pt = pt8pool.tile([P, NT, CHUNK], F8, tag="pt8")
            for t0 in range(0, ntiles, G):
                ng = min(G, ntiles - t0)
                sc = psum_sc.tile([P, G, CHUNK], F32, tag="sc")
                for idx in range(ng):
                    t = t0 + idx
                    r = t - TPC * c
                    if r >= 0:
                        off = P * r
                        nc.tensor.matmul(
                            sc[:, idx, off:],
                            kT_sb[:, t * P : (t + 1) * P],
                            qT_sb[:, c * CHUNK + off : (c + 1) * CHUNK],
                            start=True,
                            stop=not (c == 0),
                        )
                        if c == 0:
                            nc.tensor.matmul(
                                sc[:, idx, off : off + P],
                                i_sb[:],
                                m_sb[:],
                                start=False,
                                stop=True,
                            )
                    else:
                        nc.tensor.matmul(
                            sc[:, idx, :],
                            kT_sb[:, t * P : (t + 1) * P],
                            qT_sb[:, c * CHUNK : (c + 1) * CHUNK],
                            start=True,
                            stop=True,
                        )
                nfull = sum(1 for idx in range(ng) if (t0 + idx) < TPC * c)
                if nfull:
                    n = nfull * CHUNK
                    eng = pick(ACT_FIX + ACT_PER * n, DVE_FIX + DVE_PER * n)
                    if eng == "s":
                        nc.scalar.activation(
                            pt[:, t0 : t0 + nfull, :],
                            sc[:, :nfull, :],
                            AF.Exp,
                            scale=SCALE,
                            bias=nbias[:],
                        )
                    else:
                        nc.vector.tensor_scalar(
                            pt[:, t0 : t0 + nfull, :].bitcast(U8),
                            sc[:, :nfull, :],
                            A_U8,
                            B_U8,
                            AluOpType.mult,
                            AluOpType.add,
                        )
                for idx in range(nfull, ng):
                    t = t0 + idx
                    r = t - TPC * c
                    off = P * r
                    n = CHUNK - off
                    if c == 0:
                        est["s"] += ACT_FIX + ACT_PER * n
                        nc.scalar.activation(
                            pt[:, t, off:],
                            sc[:, idx, off:],
                            AF.Exp,
                            scale=SCALE,
                            bias=nbias[:],
                        )
                    else:
                        est["v"] += DVE_FIX + DVE_PER * n
                        nc.vector.scalar_tensor_tensor(
                            pt[:, t, off:].bitcast(U8),
                            sc[:, idx, off:],
                            A_U8,
                            mask2_sb[:, : CHUNK - off],
                            AluOpType.mult,
                            AluOpType.add,
                        )
                if work:
                    work.popleft()()
            return pt

        def make_av_thunks(h, c, pt):
            """AV for one (head, chunk) as thunks: j-subtile accumulation
            chains into the 2-bank av PSUM pair, with a converting copy after
            each pair of chains and one DMA at the end. Emitted interleaved
            between the NEXT chunk's QK groups."""
            av = psum_av.tile([P, 2, CHUNK], F32, tag="av")
            o_ext = opool.tile([P, TPC, VW], F16, tag="o")
            thunks = deque()

            def jchain(j, half=None):
                """half=0/1 splits the accumulation chain for finer PE
                interleaving (half 0 emits start, half 1 emits stop)."""
                nk = TPC * c + j + 1
                slot = av[:, j % 2, :VW]
                if c == 0:
                    for t in range(nk):
                        nc.tensor.matmul(
                            slot,
                            pt[:, t, j * P : (j + 1) * P],
                            v16_sb[:, t, :],
                            start=(t == 0),
                            stop=(t == nk - 1),
                        )
                    return
                npair = nk // 2
                odd = nk % 2
                mid = (npair + 1) // 2
                rng = (
                    range(npair)
                    if half is None
                    else (range(mid) if half == 0 else range(mid, npair))
                )
                for m in rng:
                    nc.tensor.matmul(
                        slot,
                        pt[:, 2 * m : 2 * m + 2, j * P : (j + 1) * P],
                        v8_sb[:, 2 * m : 2 * m + 2, :],
                        start=(m == 0),
                        stop=(m == npair - 1 and not odd),
                        perf_mode=mybir.MatmulPerfMode.DoubleRow,
                    )
                if odd and half != 0:
                    nc.tensor.matmul(
                        slot,
                        pt[:, nk - 1, j * P : (j + 1) * P],
                        v8_sb[:, nk - 1, :],
                        start=(nk == 1),
                        stop=True,
                    )

            def copy_pair(jp):
                n = 2 * VW
                eng = pick(ACT_FIX + ACT_PER * n, DVE_FIX + DVE_PER * n)
                if eng == "s":
                    nc.scalar.copy(o_ext[:, 2 * jp : 2 * jp + 2, :], av[:, :, :VW])
                else:
                    nc.vector.tensor_scalar_mul(
                        o_ext[:, 2 * jp : 2 * jp + 2, :], av[:, :, :VW], 1.0
                    )

            def fin():
                copy_pair(1)
                nc.sync.dma_start(
                    ox[h, c * CHUNK : (c + 1) * CHUNK, :].rearrange(
                        "(j p) w -> p j w", p=P
                    ),
                    o_ext[:],
                )

            thunks.append(lambda: jchain(0))
            thunks.append(lambda: jchain(1))
            thunks.append(lambda: copy_pair(0))
            thunks.append(lambda: jchain(2))
            thunks.append(lambda: jchain(3))
            thunks.append(fin)
            return thunks

        # Alternate ascending/descending chunk order per head: compute can
        # start as soon as the first (h0, c0) input slices land, every big
        # c3-QK start is cushioned by the previous head's big c3-AV (and
        # small c0-QK starts follow tiny c0-AVs), and the kernel tail is the
        # SHORT chunk-0 AV.
        pending = deque()
        for h in range(HL):
            order = range(NCH) if h == 0 else reversed(range(NCH))
            for c in order:
                # (h0, c1): don't interleave chunk-0's AV into the QK groups —
                # those thunks wait on the late-arriving v16 DMA and would
                # stall the PE mid-QK; drained after the groups instead.
                work = deque() if (h, c) == (0, 1) else pending
                pt = emit_qk_exp(h, c, work)
                while pending:
                    pending.popleft()()
                pending = make_av_thunks(h, c, pt)
                if h == 0 and c <= 2:
                    late_dmas(c)
        while pending:
            pending.popleft()()

    split_multi_waits(nc)
    return nc


def _make_mask() -> np.ndarray:
    kp = np.arange(P)[:, None]
    n = np.arange(P)[None, :]
    return np.where(kp > n, NEG, 0.0).astype(np.float16)


def _make_mask2() -> np.ndarray:
    p = np.arange(P)[:, None]
    u = np.arange(CHUNK)[None, :]
    return np.where(u >= p, B_U8, MASK_NEG).astype(np.float16)


def core_inputs(q, k, v, core):
    h0 = core * HL
    qTh = np.ascontiguousarray(q[:, h0 : h0 + HL, :].transpose(1, 2, 0)).astype(
        np.float16
    )
    kTh = np.ascontiguousarray(k[:, core, :].T).astype(np.float16)
    vxh = np.zeros((S, VW), dtype=np.float32)
    vxh[:, :D] = v[:, core, :]
    vxh[:, D] = 1.0
    return {
        "qT": qTh,
        "kT": kTh,
        "vx8": vxh.astype(NPF8),
        "vx16": vxh[:CHUNK].astype(np.float16),
        "mask": _make_mask(),
        "ident": np.eye(P, dtype=np.float16),
        "mask2": _make_mask2(),
    }


_NC = None


def _get_nc():
    global _NC
    if _NC is None:
        _NC = build_nc()
    return _NC


def make_in_maps(q, k, v):
    return [core_inputs(q, k, v, c) for c in range(N_CORES)]


def run(in_maps, **kwargs):
    return run_bass_kernel_spmd(_get_nc(), in_maps, list(range(N_CORES)), **kwargs)


def kernel(q: np.ndarray, k: np.ndarray, v: np.ndarray) -> np.ndarray:
    q = np.asarray(q, dtype=np.float32)
    k = np.asarray(k, dtype=np.float32)
    v = np.asarray(v, dtype=np.float32)
    res = run(make_in_maps(q, k, v))
    out = np.empty((S, N_CORES * HL * D), dtype=np.float32)
    for core in range(N_CORES):
        oxc = np.asarray(res.results[core]["ox"], dtype=np.float32)  # [HL,S,VW]
        for h in range(HL):
            col = (core * HL + h) * D
            out[:, col : col + D] = oxc[h, :, :D] / oxc[h, :, D : D + 1]
    return out


# revision 50
# speedup vs baseline: 1.0277x; 1.0277x over previous
"""Causal GQA attention (S=2048, Hq=32, Hkv=8, D=128, fp32 IO) on 8 Trainium2
NeuronCores, sharded over heads: core i handles q-heads 4i..4i+3 and kv-head i
(no cross-core communication).

Per-core Bass/Tile kernel design (v2 — fp8 + dual-engine exp):
- Scores are computed TRANSPOSED (S^T[k, q], d on the contraction partitions)
  from host-pre-transposed fp16 Q/K, exactly as the fp16 baseline.
- The exp is split across BOTH the scalar engine (native Exp activation with a
  folded -1 bias, fp8e4 output) and the vector engine (Schraudolph fast-exp:
  one tensor_scalar computing round(score*A + B) into a uint8 tile whose bytes
  ARE the fp8e4 encoding of 2^((u-56)/8) ~= exp(score*SCALE - 1); the fp32->u8
  conversion saturates at 0/255 and rounds-to-nearest, HW-verified). Causal
  triangle masking for DVE-handled diagonal tiles is folded into the same op
  via scalar_tensor_tensor with a precomputed additive mask (-1e9 above the
  diagonal -> u8 0 -> +0.0 in fp8).
- The P*V accumulation runs in fp8 with MatmulPerfMode.DoubleRow: each matmul
  contracts TWO 128-row k-tiles (stationary = paired P^T tiles, moving =
  paired V tiles extended with a ones column that yields the softmax
  denominator for free in column 128).
- Early queries (chunk 0, q < 512) have too few softmax terms to average away
  fp8 noise, so chunk 0 runs the original fp16 path (fp16 pt, fp16 V, scalar
  exp, -30000 identity-matmul triangle masking).
- The AV PSUM quad [128, 4, 512] is copied to SBUF fp16 UNNORMALIZED (with the
  denominator riding in column 128) and DMA'd out; the host does the divide.
"""

from collections import deque
from contextlib import ExitStack

import numpy as np

import concourse.bass as bass
import concourse.mybir as mybir
import concourse.tile as tile
from concourse.mybir import ActivationFunctionType as AF
from concourse.alu_op_type import AluOpType
from concourse.vector_clock import ScopedClock
from concourse.bass_utils import run_bass_kernel_spmd

# Walrus's BIR-simulation pass is ~85% of NEFF compile time and is a
# verification-only pass; skip it.
try:
    import concourse.bass_utils as _bu

    if not getattr(_bu, "_birsim_patched", False):
        _orig_run_command = _bu.run_command

        def _fast_run_command(cmd, *a, **kw):
            cmd = [
                c.replace("--enable-birsim=true", "--enable-birsim=false")
                if isinstance(c, str)
                else c
                for c in cmd
            ]
            return _orig_run_command(cmd, *a, **kw)

        _bu.run_command = _fast_run_command
        _bu._birsim_patched = True
except Exception:
    pass

S = 2048
D = 128
P = 128
NT = S // P          # 16 k-tiles
CHUNK = 512          # q columns per score chunk
NCH = S // CHUNK     # 4 chunks
TPC = CHUNK // P     # 4 k-tiles / diag rows per chunk
VW = 132             # v_ext free width (128 d + 1 ones + 3 pad)
G = 2                # k-tiles per PSUM score group (2 banks; av quad takes 4)
SCALE = 0.08838834764831845
NEG = -30000.0
HL = 4               # q-heads per core
N_CORES = 8

LN2 = 0.6931471805599453
C_SHIFT = 1.0                      # exp(s*SCALE - C): keeps fp8 range safe
A_U8 = 8.0 * SCALE / LN2           # Schraudolph multiplier
B_U8 = 56.0 - 0.52 - 8.0 * C_SHIFT / LN2   # bias incl. -0.52 centering
MASK_NEG = -60000.0                # fp16-representable; a*s+MASK_NEG -> u8 0

F16 = mybir.dt.float16
F32 = mybir.dt.float32
F8 = mybir.dt.float8e4
U8 = mybir.dt.uint8
NPF8 = mybir.dt.np(F8)

WAIT_LIMIT = 1  # this image's walrus encodes at most one sync-wait per inst

# Engine-balance cost model (ns); static greedy assignment. HW-calibrated:
# scalar ACTIVATE = n*0.833 + 293; DVE tensor_scalar/stt = n*1.042 + 190
# (independent of output dtype / bias).
ACT_FIX, ACT_PER = 293.0, 0.833
DVE_FIX, DVE_PER = 190.0, 1.042


class SplitDrainTileContext(tile.TileContext):
    """TileContext whose exit drain spreads its semaphore waits over
    multiple SP instructions (walrus here caps sync-waits per inst)."""

    def _drain_and_barrier(self, tick_clock, wait_clock):
        drain_inst = self.nc.sync.drain()
        wait_clock.add_sem_waits(
            drain_inst.ins, ScopedClock({None: tick_clock.global_clock})
        )
        waits = list(drain_inst.ins.sync_info.on_wait)
        if len(waits) > WAIT_LIMIT:
            drain_inst.ins.sync_info = mybir.SyncInfo(
                on_wait=waits[:WAIT_LIMIT],
                on_update=list(drain_inst.ins.sync_info.on_update),
            )
            for i in range(WAIT_LIMIT, len(waits), WAIT_LIMIT):
                nop = self.nc.sync.nop(nofuse=True)
                nop.ins.sync_info = mybir.SyncInfo(
                    on_wait=waits[i : i + WAIT_LIMIT], on_update=[]
                )
        self.nc.all_engine_barrier()
        popped = self.nc._tile_sem_poison_stack.pop()
        assert popped is self._sem_poison
        self.nc.clear_and_free_semaphores(list(self.sems.allocated().values()))
        self.nc.all_engine_barrier()


def split_multi_waits(nc, limit: int = WAIT_LIMIT):
    """Spread >limit sync-waits onto same-engine NOPs inserted before the
    instruction (engines execute in order: cumulative semantics identical)."""
    n_split = 0
    for fn in nc.m.functions:
        for bb in fn.blocks:
            out = []
            changed = False
            for inst in bb.instructions:
                si = inst.sync_info
                waits = list(si.on_wait) if si is not None else []
                if len(waits) > limit:
                    changed = True
                    n_split += 1
                    extra = waits[:-limit]
                    for ci in range(0, len(extra), limit):
                        nop = mybir.InstNoOp(
                            name=f"{inst.name}-sw{ci}", ins=[], outs=[]
                        )
                        nop.engine = inst.engine
                        nop.sync_info = mybir.SyncInfo(
                            on_wait=extra[ci : ci + limit], on_update=[]
                        )
                        nc.register_instruction(nop, overwrite=True)
                        out.append(nop)
                    inst.sync_info = mybir.SyncInfo(
                        on_wait=waits[-limit:], on_update=list(si.on_update)
                    )
                out.append(inst)
            if changed:
                bb.instructions = out
    return n_split


def build_nc() -> bass.Bass:
    nc = bass.Bass()

    qT = nc.dram_tensor("qT", [HL, P, S], F16, kind="ExternalInput")
    kT = nc.dram_tensor("kT", [P, S], F16, kind="ExternalInput")
    vx8 = nc.dram_tensor("vx8", [S, VW], F8, kind="ExternalInput")
    vx16 = nc.dram_tensor("vx16", [CHUNK, VW], F16, kind="ExternalInput")
    mask = nc.dram_tensor("mask", [P, P], F16, kind="ExternalInput")
    ident = nc.dram_tensor("ident", [P, P], F16, kind="ExternalInput")
    # mask2[p, u] = B_U8 if u >= p else MASK_NEG; diag tile r at window
    # [off:] uses mask2[:, :CHUNK-off] (value depends only on j-off-p).
    mask2 = nc.dram_tensor("mask2", [P, CHUNK], F16, kind="ExternalInput")
    ox = nc.dram_tensor("ox", [HL, S, VW], F16, kind="ExternalOutput")

    # running per-engine cost estimates for static load balancing
    est = {"s": 0.0, "v": 0.0}

    def pick(scalar_cost, dve_cost):
        if est["s"] + scalar_cost <= est["v"] + dve_cost:
            est["s"] += scalar_cost
            return "s"
        est["v"] += dve_cost
        return "v"

    with SplitDrainTileContext(nc) as tc, ExitStack() as ctx:
        const = ctx.enter_context(tc.tile_pool(name="const", bufs=1))
        qpool = ctx.enter_context(tc.tile_pool(name="qpool", bufs=HL))
        pt8pool = ctx.enter_context(tc.tile_pool(name="pt8pool", bufs=2))
        pt16pool = ctx.enter_context(tc.tile_pool(name="pt16pool", bufs=2))
        opool = ctx.enter_context(tc.tile_pool(name="opool", bufs=2))
        psum_sc = ctx.enter_context(tc.tile_pool(name="psc", bufs=3, space="PSUM"))
        psum_av = ctx.enter_context(tc.tile_pool(name="pav", bufs=1, space="PSUM"))

        # Head 0 runs chunks ASCENDING, so the kernel can start computing
        # (h0, c0) after only kT[:, :512] + qT0[:, :512] + the tiny c0-path
        # consts have landed. Input DMAs are split across the sync and gpsimd
        # queues in 512-col pieces ordered by first use.
        kT_sb = const.tile([P, S], F16)
        qT_sbs = []
        qT_sb0 = qpool.tile([P, S], F16, tag="q")
        qT_sbs.append(qT_sb0)
        m_sb = const.tile([P, P], F16)
        i_sb = const.tile([P, P], F16)
        v16_sb = const.tile([P, TPC, VW], F16)
        v8_sb = const.tile([P, NT, VW], F8)
        mask2_sb = const.tile([P, CHUNK], F16)
        nbias = const.tile([P, 1], F32)

        # gpsimd issues NO DMAs (its DGE drain at kernel exit costs ~3.5us
        # once used). Early small pieces ride the scalar queue before its
        # first exp can run; the late bulk (v8, qT0 tail, qT2) is emitted
        # MID-PROGRAM on the scalar queue (see the main loop) where the
        # engine has slack, with est[] charged for the issue time.
        nc.sync.dma_start(kT_sb[:, :CHUNK], kT[:, :CHUNK])
        nc.scalar.dma_start(qT_sb0[:, :CHUNK], qT[0, :, :CHUNK])
        nc.sync.dma_start(m_sb[:], mask[:])
        nc.sync.dma_start(i_sb[:], ident[:])
        nc.vector.memset(nbias[:], -C_SHIFT)
        nc.scalar.dma_start(
            qT_sb0[:, CHUNK : 2 * CHUNK], qT[0, :, CHUNK : 2 * CHUNK]
        )
        nc.sync.dma_start(kT_sb[:, CHUNK : 2 * CHUNK], kT[:, CHUNK : 2 * CHUNK])
        nc.scalar.dma_start(mask2_sb[:], mask2[:])
        nc.sync.dma_start(v16_sb[:], vx16.rearrange("(t p) w -> p t w", p=P))
        nc.sync.dma_start(kT_sb[:, 2 * CHUNK :], kT[:, 2 * CHUNK :])
        for h in range(1, HL):
            qT_sb = qpool.tile([P, S], F16, tag="q")
            if h != 2:
                nc.sync.dma_start(qT_sb[:], qT[h])
            qT_sbs.append(qT_sb)

        def late_dmas(step):
            """Mid-program input DMAs on the scalar queue."""
            if step == 0:
                est["s"] += 1100.0
                nc.scalar.dma_start(
                    v8_sb[:], vx8.rearrange("(t p) w -> p t w", p=P)
                )
                est["s"] += 900.0
                nc.scalar.dma_start(
                    qT_sb0[:, 2 * CHUNK :], qT[0, :, 2 * CHUNK :]
                )
            elif step == 1:
                est["s"] += 1700.0
                nc.scalar.dma_start(qT_sbs[2][:], qT[2])

        def emit_qk_exp(h, c, work):
            """QK + exp for one (head, chunk). After each PSUM score group is
            emitted, one pending AV thunk from the previous chunk is emitted
            so the PE interleaves AV matmuls with QK instead of stalling on
            the exp engines draining score PSUM (stalls also drop the PE out
            of its fast p-state).

            c == 0: fp16 path (trimmed QK + identity-matmul triangle mask,
            scalar exp -> fp16 pt). c >= 1: fp8 path (no mask matmuls;
            full-tile groups exp'd on scalar Exp->fp8 or DVE Schraudolph->u8;
            diagonal tiles on DVE scalar_tensor_tensor with the mask4
            additive table)."""
            qT_sb = qT_sbs[h]
            ntiles = TPC * (c + 1)
            if c == 0:
                pt = pt16pool.tile([P, TPC, CHUNK], F16, tag="pt16")
            else:
                pt = pt8pool.tile([P, NT, CHUNK], F8, tag="pt8")
            for t0 in range(0, ntiles, G):
                ng = min(G, ntiles - t0)
                sc = psum_sc.tile([P, G, CHUNK], F32, tag="sc")
                for idx in range(ng):
                    t = t0 + idx
                    r = t - TPC * c
                    if r >= 0:
                        off = P * r
                        nc.tensor.matmul(
                            sc[:, idx, off:],
                            kT_sb[:, t * P : (t + 1) * P],
                            qT_sb[:, c * CHUNK + off : (c + 1) * CHUNK],
                            start=True,
                            stop=not (c == 0),
                        )
                        if c == 0:
                            nc.tensor.matmul(
                                sc[:, idx, off : off + P],
                                i_sb[:],
                                m_sb[:],
                                start=False,
                                stop=True,
                            )
                    else:
                        nc.tensor.matmul(
                            sc[:, idx, :],
                            kT_sb[:, t * P : (t + 1) * P],
                            qT_sb[:, c * CHUNK : (c + 1) * CHUNK],
                            start=True,
                            stop=True,
                        )
                nfull = sum(1 for idx in range(ng) if (t0 + idx) < TPC * c)
                if nfull:
                    n = nfull * CHUNK
                    eng = pick(ACT_FIX + ACT_PER * n, DVE_FIX + DVE_PER * n)
                    if eng == "s":
                        nc.scalar.activation(
                            pt[:, t0 : t0 + nfull, :],
                            sc[:, :nfull, :],
                            AF.Exp,
                            scale=SCALE,
                            bias=nbias[:],
                        )
                    else:
                        nc.vector.tensor_scalar(
                            pt[:, t0 : t0 + nfull, :].bitcast(U8),
                            sc[:, :nfull, :],
                            A_U8,
                            B_U8,
                            AluOpType.mult,
                            AluOpType.add,
                        )
                for idx in range(nfull, ng):
                    t = t0 + idx
                    r = t - TPC * c
                    off = P * r
                    n = CHUNK - off
                    if c == 0:
                        est["s"] += ACT_FIX + ACT_PER * n
                        nc.scalar.activation(
                            pt[:, t, off:],
                            sc[:, idx, off:],
                            AF.Exp,
                            scale=SCALE,
                            bias=nbias[:],
                        )
                    else:
                        est["v"] += DVE_FIX + DVE_PER * n
                        nc.vector.scalar_tensor_tensor(
                            pt[:, t, off:].bitcast(U8),
                            sc[:, idx, off:],
                            A_U8,
                            mask2_sb[:, : CHUNK - off],
                            AluOpType.mult,
                            AluOpType.add,
                        )
                if work:
                    work.popleft()()
            return pt

        def make_av_thunks(h, c, pt):
            """AV for one (head, chunk) as thunks: j-subtile accumulation
            chains into the 2-bank av PSUM pair, with a converting copy after
            each pair of chains and one DMA at the end. Emitted interleaved
            between the NEXT chunk's QK groups."""
            av = psum_av.tile([P, 2, CHUNK], F32, tag="av")
            o_ext = opool.tile([P, TPC, VW], F16, tag="o")
            thunks = deque()

            def jchain(j, half=None):
                """half=0/1 splits the accumulation chain for finer PE
                interleaving (half 0 emits start, half 1 emits stop)."""
                nk = TPC * c + j + 1
                slot = av[:, j % 2, :VW]
                if c == 0:
                    for t in range(nk):
                        nc.tensor.matmul(
                            slot,
                            pt[:, t, j * P : (j + 1) * P],
                            v16_sb[:, t, :],
                            start=(t == 0),
                            stop=(t == nk - 1),
                        )
                    return
                npair = nk // 2
                odd = nk % 2
                mid = (npair + 1) // 2
                rng = (
                    range(npair)
                    if half is None
                    else (range(mid) if half == 0 else range(mid, npair))
                )
                for m in rng:
                    nc.tensor.matmul(
                        slot,
                        pt[:, 2 * m : 2 * m + 2, j * P : (j + 1) * P],
                        v8_sb[:, 2 * m : 2 * m + 2, :],
                        start=(m == 0),
                        stop=(m == npair - 1 and not odd),
                        perf_mode=mybir.MatmulPerfMode.DoubleRow,
                    )
                if odd and half != 0:
                    nc.tensor.matmul(
                        slot,
                        pt[:, nk - 1, j * P : (j + 1) * P],
                        v8_sb[:, nk - 1, :],
                        start=(nk == 1),
                        stop=True,
                    )

            def copy_pair(jp):
                n = 2 * VW
                eng = pick(ACT_FIX + ACT_PER * n, DVE_FIX + DVE_PER * n)
                if eng == "s":
                    nc.scalar.copy(o_ext[:, 2 * jp : 2 * jp + 2, :], av[:, :, :VW])
                else:
                    nc.vector.tensor_scalar_mul(
                        o_ext[:, 2 * jp : 2 * jp + 2, :], av[:, :, :VW], 1.0
                    )

            def fin():
                copy_pair(1)
                nc.sync.dma_start(
                    ox[h, c * CHUNK : (c + 1) * CHUNK, :].rearrange(
                        "(j p) w -> p j w", p=P
                    ),
                    o_ext[:],
                )

            thunks.append(lambda: jchain(0))
            thunks.append(lambda: jchain(1))
            thunks.append(lambda: copy_pair(0))
            thunks.append(lambda: jchain(2))
            thunks.append(lambda: jchain(3))
            thunks.append(fin)
            return thunks

        # Alternate ascending/descending chunk order per head: compute can
        # start as soon as the first (h0, c0) input slices land, every big
        # c3-QK start is cushioned by the previous head's big c3-AV (and
        # small c0-QK starts follow tiny c0-AVs), and the kernel tail is the
        # SHORT chunk-0 AV.
        # AV thunks carry across chunk AND head boundaries: before each QK
        # emission, pending is drained down to at most ONE chunk's worth (the
        # newest — FIFO pops emit older thunks first), so the next head's big
        # c3-QK start keeps an interleave cushion instead of running bare.
        # The <= 6 cap also guarantees AV(h, c-2) is fully emitted before
        # exp(h, c) reuses its pt pool buffer (bufs=2 WAR safety).
        pending = deque()
        for h in range(HL):
            order = range(NCH) if h == 0 else reversed(range(NCH))
            for c in order:
                while len(pending) > 6:
                    pending.popleft()()
                # (h0, c1): don't interleave chunk-0's AV into the QK groups —
                # those thunks wait on the late-arriving v16 DMA and would
                # stall the PE mid-QK; drained after the groups instead.
                work = deque() if (h, c) == (0, 1) else pending
                pt = emit_qk_exp(h, c, work)
                pending.extend(make_av_thunks(h, c, pt))
                if h == 0 and c == 0:
                    late_dmas(0)
                elif h == 0 and c == 1:
                    late_dmas(1)
        while pending:
            pending.popleft()()

    split_multi_waits(nc)
    return nc


def _make_mask() -> np.ndarray:
    kp = np.arange(P)[:, None]
    n = np.arange(P)[None, :]
    return np.where(kp > n, NEG, 0.0).astype(np.float16)


def _make_mask2() -> np.ndarray:
    p = np.arange(P)[:, None]
    u = np.arange(CHUNK)[None, :]
    return np.where(u >= p, B_U8, MASK_NEG).astype(np.float16)


def core_inputs(q, k, v, core):
    h0 = core * HL
    qTh = np.ascontiguousarray(q[:, h0 : h0 + HL, :].transpose(1, 2, 0)).astype(
        np.float16
    )
    kTh = np.ascontiguousarray(k[:, core, :].T).astype(np.float16)
    vxh = np.zeros((S, VW), dtype=np.float32)
    vxh[:, :D] = v[:, core, :]
    vxh[:, D] = 1.0
    return {
        "qT": qTh,
        "kT": kTh,
        "vx8": vxh.astype(NPF8),
        "vx16": vxh[:CHUNK].astype(np.float16),
        "mask": _make_mask(),
        "ident": np.eye(P, dtype=np.float16),
        "mask2": _make_mask2(),
    }


_NC = None


def _get_nc():
    global _NC
    if _NC is None:
        _NC = build_nc()
    return _NC


def make_in_maps(q, k, v):
    return [core_inputs(q, k, v, c) for c in range(N_CORES)]


def run(in_maps, **kwargs):
    return run_bass_kernel_spmd(_get_nc(), in_maps, list(range(N_CORES)), **kwargs)


def kernel(q: np.ndarray, k: np.ndarray, v: np.ndarray) -> np.ndarray:
    q = np.asarray(q, dtype=np.float32)
    k = np.asarray(k, dtype=np.float32)
    v = np.asarray(v, dtype=np.float32)
    res = run(make_in_maps(q, k, v))
    out = np.empty((S, N_CORES * HL * D), dtype=np.float32)
    for core in range(N_CORES):
        oxc = np.asarray(res.results[core]["ox"], dtype=np.float32)  # [HL,S,VW]
        for h in range(HL):
            col = (core * HL + h) * D
            out[:, col : col + D] = oxc[h, :, :D] / oxc[h, :, D : D + 1]
    return out


# revision 51
# speedup vs baseline: 1.0282x; 1.0005x over previous
"""Causal GQA attention (S=2048, Hq=32, Hkv=8, D=128, fp32 IO) on 8 Trainium2
NeuronCores, sharded over heads: core i handles q-heads 4i..4i+3 and kv-head i
(no cross-core communication).

Per-core Bass/Tile kernel design (v2 — fp8 + dual-engine exp):
- Scores are computed TRANSPOSED (S^T[k, q], d on the contraction partitions)
  from host-pre-transposed fp16 Q/K, exactly as the fp16 baseline.
- The exp is split across BOTH the scalar engine (native Exp activation with a
  folded -1 bias, fp8e4 output) and the vector engine (Schraudolph fast-exp:
  one tensor_scalar computing round(score*A + B) into a uint8 tile whose bytes
  ARE the fp8e4 encoding of 2^((u-56)/8) ~= exp(score*SCALE - 1); the fp32->u8
  conversion saturates at 0/255 and rounds-to-nearest, HW-verified). Causal
  triangle masking for DVE-handled diagonal tiles is folded into the same op
  via scalar_tensor_tensor with a precomputed additive mask (-1e9 above the
  diagonal -> u8 0 -> +0.0 in fp8).
- The P*V accumulation runs in fp8 with MatmulPerfMode.DoubleRow: each matmul
  contracts TWO 128-row k-tiles (stationary = paired P^T tiles, moving =
  paired V tiles extended with a ones column that yields the softmax
  denominator for free in column 128).
- Early queries (chunk 0, q < 512) have too few softmax terms to average away
  fp8 noise, so chunk 0 runs the original fp16 path (fp16 pt, fp16 V, scalar
  exp, -30000 identity-matmul triangle masking).
- The AV PSUM quad [128, 4, 512] is copied to SBUF fp16 UNNORMALIZED (with the
  denominator riding in column 128) and DMA'd out; the host does the divide.
"""

from collections import deque
from contextlib import ExitStack

import numpy as np

import concourse.bass as bass
import concourse.mybir as mybir
import concourse.tile as tile
from concourse.mybir import ActivationFunctionType as AF
from concourse.alu_op_type import AluOpType
from concourse.vector_clock import ScopedClock
from concourse.bass_utils import run_bass_kernel_spmd

# Walrus's BIR-simulation pass is ~85% of NEFF compile time and is a
# verification-only pass; skip it.
try:
    import concourse.bass_utils as _bu

    if not getattr(_bu, "_birsim_patched", False):
        _orig_run_command = _bu.run_command

        def _fast_run_command(cmd, *a, **kw):
            cmd = [
                c.replace("--enable-birsim=true", "--enable-birsim=false")
                if isinstance(c, str)
                else c
                for c in cmd
            ]
            return _orig_run_command(cmd, *a, **kw)

        _bu.run_command = _fast_run_command
        _bu._birsim_patched = True
except Exception:
    pass

S = 2048
D = 128
P = 128
NT = S // P          # 16 k-tiles
CHUNK = 512          # q columns per score chunk
NCH = S // CHUNK     # 4 chunks
TPC = CHUNK // P     # 4 k-tiles / diag rows per chunk
VW = 132             # v_ext free width (128 d + 1 ones + 3 pad)
G = 2                # k-tiles per PSUM score group (2 banks; av quad takes 4)
SCALE = 0.08838834764831845
NEG = -30000.0
HL = 4               # q-heads per core
N_CORES = 8

LN2 = 0.6931471805599453
C_SHIFT = 1.0                      # exp(s*SCALE - C): keeps fp8 range safe
A_U8 = 8.0 * SCALE / LN2           # Schraudolph multiplier
B_U8 = 56.0 - 0.52 - 8.0 * C_SHIFT / LN2   # bias incl. -0.52 centering
MASK_NEG = -60000.0                # fp16-representable; a*s+MASK_NEG -> u8 0

F16 = mybir.dt.float16
F32 = mybir.dt.float32
F8 = mybir.dt.float8e4
U8 = mybir.dt.uint8
NPF8 = mybir.dt.np(F8)

WAIT_LIMIT = 1  # this image's walrus encodes at most one sync-wait per inst

# Engine-balance cost model (ns); static greedy assignment. HW-calibrated:
# scalar ACTIVATE = n*0.833 + 293; DVE tensor_scalar/stt = n*1.042 + 190
# (independent of output dtype / bias).
ACT_FIX, ACT_PER = 293.0, 0.833
DVE_FIX, DVE_PER = 190.0, 1.042


class SplitDrainTileContext(tile.TileContext):
    """TileContext whose exit drain spreads its semaphore waits over
    multiple SP instructions (walrus here caps sync-waits per inst)."""

    def _drain_and_barrier(self, tick_clock, wait_clock):
        drain_inst = self.nc.sync.drain()
        wait_clock.add_sem_waits(
            drain_inst.ins, ScopedClock({None: tick_clock.global_clock})
        )
        waits = list(drain_inst.ins.sync_info.on_wait)
        if len(waits) > WAIT_LIMIT:
            drain_inst.ins.sync_info = mybir.SyncInfo(
                on_wait=waits[:WAIT_LIMIT],
                on_update=list(drain_inst.ins.sync_info.on_update),
            )
            for i in range(WAIT_LIMIT, len(waits), WAIT_LIMIT):
                nop = self.nc.sync.nop(nofuse=True)
                nop.ins.sync_info = mybir.SyncInfo(
                    on_wait=waits[i : i + WAIT_LIMIT], on_update=[]
                )
        self.nc.all_engine_barrier()
        popped = self.nc._tile_sem_poison_stack.pop()
        assert popped is self._sem_poison
        self.nc.clear_and_free_semaphores(list(self.sems.allocated().values()))
        self.nc.all_engine_barrier()


def split_multi_waits(nc, limit: int = WAIT_LIMIT):
    """Spread >limit sync-waits onto same-engine NOPs inserted before the
    instruction (engines execute in order: cumulative semantics identical)."""
    n_split = 0
    for fn in nc.m.functions:
        for bb in fn.blocks:
            out = []
            changed = False
            for inst in bb.instructions:
                si = inst.sync_info
                waits = list(si.on_wait) if si is not None else []
                if len(waits) > limit:
                    changed = True
                    n_split += 1
                    extra = waits[:-limit]
                    for ci in range(0, len(extra), limit):
                        nop = mybir.InstNoOp(
                            name=f"{inst.name}-sw{ci}", ins=[], outs=[]
                        )
                        nop.engine = inst.engine
                        nop.sync_info = mybir.SyncInfo(
                            on_wait=extra[ci : ci + limit], on_update=[]
                        )
                        nc.register_instruction(nop, overwrite=True)
                        out.append(nop)
                    inst.sync_info = mybir.SyncInfo(
                        on_wait=waits[-limit:], on_update=list(si.on_update)
                    )
                out.append(inst)
            if changed:
                bb.instructions = out
    return n_split


def build_nc() -> bass.Bass:
    nc = bass.Bass()

    qT = nc.dram_tensor("qT", [HL, P, S], F16, kind="ExternalInput")
    kT = nc.dram_tensor("kT", [P, S], F16, kind="ExternalInput")
    vx8 = nc.dram_tensor("vx8", [S, VW], F8, kind="ExternalInput")
    vx16 = nc.dram_tensor("vx16", [CHUNK, VW], F16, kind="ExternalInput")
    mask = nc.dram_tensor("mask", [P, P], F16, kind="ExternalInput")
    ident = nc.dram_tensor("ident", [P, P], F16, kind="ExternalInput")
    # mask2[p, u] = B_U8 if u >= p else MASK_NEG; diag tile r at window
    # [off:] uses mask2[:, :CHUNK-off] (value depends only on j-off-p).
    mask2 = nc.dram_tensor("mask2", [P, CHUNK], F16, kind="ExternalInput")
    ox = nc.dram_tensor("ox", [HL, S, VW], F16, kind="ExternalOutput")

    # running per-engine cost estimates for static load balancing
    est = {"s": 0.0, "v": 0.0}

    def pick(scalar_cost, dve_cost):
        if est["s"] + scalar_cost <= est["v"] + dve_cost:
            est["s"] += scalar_cost
            return "s"
        est["v"] += dve_cost
        return "v"

    with SplitDrainTileContext(nc) as tc, ExitStack() as ctx:
        const = ctx.enter_context(tc.tile_pool(name="const", bufs=1))
        qpool = ctx.enter_context(tc.tile_pool(name="qpool", bufs=HL))
        pt8pool = ctx.enter_context(tc.tile_pool(name="pt8pool", bufs=2))
        pt16pool = ctx.enter_context(tc.tile_pool(name="pt16pool", bufs=2))
        opool = ctx.enter_context(tc.tile_pool(name="opool", bufs=2))
        psum_sc = ctx.enter_context(tc.tile_pool(name="psc", bufs=3, space="PSUM"))
        psum_av = ctx.enter_context(tc.tile_pool(name="pav", bufs=1, space="PSUM"))

        # Head 0 runs chunks ASCENDING, so the kernel can start computing
        # (h0, c0) after only kT[:, :512] + qT0[:, :512] + the tiny c0-path
        # consts have landed. Input DMAs are split across the sync and gpsimd
        # queues in 512-col pieces ordered by first use.
        kT_sb = const.tile([P, S], F16)
        qT_sbs = []
        qT_sb0 = qpool.tile([P, S], F16, tag="q")
        qT_sbs.append(qT_sb0)
        m_sb = const.tile([P, P], F16)
        i_sb = const.tile([P, P], F16)
        v16_sb = const.tile([P, TPC, VW], F16)
        v8_sb = const.tile([P, NT, VW], F8)
        mask2_sb = const.tile([P, CHUNK], F16)
        nbias = const.tile([P, 1], F32)

        # gpsimd issues NO DMAs (its DGE drain at kernel exit costs ~3.5us
        # once used). Early small pieces ride the scalar queue before its
        # first exp can run; the late bulk (v8, qT0 tail, qT2) is emitted
        # MID-PROGRAM on the scalar queue (see the main loop) where the
        # engine has slack, with est[] charged for the issue time.
        nc.sync.dma_start(kT_sb[:, :CHUNK], kT[:, :CHUNK])
        nc.scalar.dma_start(qT_sb0[:, :CHUNK], qT[0, :, :CHUNK])
        nc.sync.dma_start(m_sb[:], mask[:])
        nc.sync.dma_start(i_sb[:], ident[:])
        nc.vector.memset(nbias[:], -C_SHIFT)
        nc.scalar.dma_start(
            qT_sb0[:, CHUNK : 2 * CHUNK], qT[0, :, CHUNK : 2 * CHUNK]
        )
        nc.sync.dma_start(kT_sb[:, CHUNK : 2 * CHUNK], kT[:, CHUNK : 2 * CHUNK])
        nc.scalar.dma_start(mask2_sb[:], mask2[:])
        nc.sync.dma_start(v16_sb[:], vx16.rearrange("(t p) w -> p t w", p=P))
        nc.sync.dma_start(kT_sb[:, 2 * CHUNK :], kT[:, 2 * CHUNK :])
        for h in range(1, HL):
            qT_sb = qpool.tile([P, S], F16, tag="q")
            if h != 2:
                nc.sync.dma_start(qT_sb[:], qT[h])
            qT_sbs.append(qT_sb)

        def late_dmas(step):
            """Mid-program input DMAs on the scalar queue."""
            if step == 0:
                est["s"] += 1100.0
                nc.scalar.dma_start(
                    v8_sb[:], vx8.rearrange("(t p) w -> p t w", p=P)
                )
                est["s"] += 900.0
                nc.scalar.dma_start(
                    qT_sb0[:, 2 * CHUNK :], qT[0, :, 2 * CHUNK :]
                )
            elif step == 1:
                est["s"] += 1700.0
                nc.scalar.dma_start(qT_sbs[2][:], qT[2])

        def emit_qk_exp(h, c, work):
            """QK + exp for one (head, chunk). After each PSUM score group is
            emitted, one pending AV thunk from the previous chunk is emitted
            so the PE interleaves AV matmuls with QK instead of stalling on
            the exp engines draining score PSUM (stalls also drop the PE out
            of its fast p-state).

            c == 0: fp16 path (trimmed QK + identity-matmul triangle mask,
            scalar exp -> fp16 pt). c >= 1: fp8 path (no mask matmuls;
            full-tile groups exp'd on scalar Exp->fp8 or DVE Schraudolph->u8;
            diagonal tiles on DVE scalar_tensor_tensor with the mask4
            additive table)."""
            qT_sb = qT_sbs[h]
            ntiles = TPC * (c + 1)
            if c == 0:
                pt = pt16pool.tile([P, TPC, CHUNK], F16, tag="pt16")
            else:
                pt = pt8pool.tile([P, NT, CHUNK], F8, tag="pt8")
            for t0 in range(0, ntiles, G):
                ng = min(G, ntiles - t0)
                sc = psum_sc.tile([P, G, CHUNK], F32, tag="sc")
                for idx in range(ng):
                    t = t0 + idx
                    r = t - TPC * c
                    if r >= 0:
                        off = P * r
                        nc.tensor.matmul(
                            sc[:, idx, off:],
                            kT_sb[:, t * P : (t + 1) * P],
                            qT_sb[:, c * CHUNK + off : (c + 1) * CHUNK],
                            start=True,
                            stop=not (c == 0),
                        )
                        if c == 0:
                            nc.tensor.matmul(
                                sc[:, idx, off : off + P],
                                i_sb[:],
                                m_sb[:],
                                start=False,
                                stop=True,
                            )
                    else:
                        nc.tensor.matmul(
                            sc[:, idx, :],
                            kT_sb[:, t * P : (t + 1) * P],
                            qT_sb[:, c * CHUNK : (c + 1) * CHUNK],
                            start=True,
                            stop=True,
                        )
                nfull = sum(1 for idx in range(ng) if (t0 + idx) < TPC * c)
                if nfull:
                    n = nfull * CHUNK
                    eng = pick(ACT_FIX + ACT_PER * n, DVE_FIX + DVE_PER * n)
                    if eng == "s":
                        nc.scalar.activation(
                            pt[:, t0 : t0 + nfull, :],
                            sc[:, :nfull, :],
                            AF.Exp,
                            scale=SCALE,
                            bias=nbias[:],
                        )
                    else:
                        nc.vector.tensor_scalar(
                            pt[:, t0 : t0 + nfull, :].bitcast(U8),
                            sc[:, :nfull, :],
                            A_U8,
                            B_U8,
                            AluOpType.mult,
                            AluOpType.add,
                        )
                for idx in range(nfull, ng):
                    t = t0 + idx
                    r = t - TPC * c
                    off = P * r
                    n = CHUNK - off
                    if c == 0:
                        est["s"] += ACT_FIX + ACT_PER * n
                        nc.scalar.activation(
                            pt[:, t, off:],
                            sc[:, idx, off:],
                            AF.Exp,
                            scale=SCALE,
                            bias=nbias[:],
                        )
                    else:
                        est["v"] += DVE_FIX + DVE_PER * n
                        nc.vector.scalar_tensor_tensor(
                            pt[:, t, off:].bitcast(U8),
                            sc[:, idx, off:],
                            A_U8,
                            mask2_sb[:, : CHUNK - off],
                            AluOpType.mult,
                            AluOpType.add,
                        )
                if work:
                    work.popleft()()
            return pt

        def make_av_thunks(h, c, pt):
            """AV for one (head, chunk) as thunks: j-subtile accumulation
            chains into the 2-bank av PSUM pair, with a converting copy after
            each pair of chains and one DMA at the end. Emitted interleaved
            between the NEXT chunk's QK groups."""
            av = psum_av.tile([P, 2, CHUNK], F32, tag="av")
            o_ext = opool.tile([P, TPC, VW], F16, tag="o")
            thunks = deque()

            def jchain(j, half=None):
                """half=0/1 splits the accumulation chain for finer PE
                interleaving (half 0 emits start, half 1 emits stop)."""
                nk = TPC * c + j + 1
                slot = av[:, j % 2, :VW]
                if c == 0:
                    for t in range(nk):
                        nc.tensor.matmul(
                            slot,
                            pt[:, t, j * P : (j + 1) * P],
                            v16_sb[:, t, :],
                            start=(t == 0),
                            stop=(t == nk - 1),
                        )
                    return
                npair = nk // 2
                odd = nk % 2
                mid = (npair + 1) // 2
                rng = (
                    range(npair)
                    if half is None
                    else (range(mid) if half == 0 else range(mid, npair))
                )
                for m in rng:
                    nc.tensor.matmul(
                        slot,
                        pt[:, 2 * m : 2 * m + 2, j * P : (j + 1) * P],
                        v8_sb[:, 2 * m : 2 * m + 2, :],
                        start=(m == 0),
                        stop=(m == npair - 1 and not odd),
                        perf_mode=mybir.MatmulPerfMode.DoubleRow,
                    )
                if odd and half != 0:
                    nc.tensor.matmul(
                        slot,
                        pt[:, nk - 1, j * P : (j + 1) * P],
                        v8_sb[:, nk - 1, :],
                        start=(nk == 1),
                        stop=True,
                    )

            def copy_pair(jp):
                n = 2 * VW
                eng = pick(ACT_FIX + ACT_PER * n, DVE_FIX + DVE_PER * n)
                if eng == "s":
                    nc.scalar.copy(o_ext[:, 2 * jp : 2 * jp + 2, :], av[:, :, :VW])
                else:
                    nc.vector.tensor_scalar_mul(
                        o_ext[:, 2 * jp : 2 * jp + 2, :], av[:, :, :VW], 1.0
                    )

            def fin():
                copy_pair(1)
                nc.sync.dma_start(
                    ox[h, c * CHUNK : (c + 1) * CHUNK, :].rearrange(
                        "(j p) w -> p j w", p=P
                    ),
                    o_ext[:],
                )

            thunks.append(lambda: jchain(0))
            thunks.append(lambda: jchain(1))
            thunks.append(lambda: copy_pair(0))
            thunks.append(lambda: jchain(2))
            thunks.append(lambda: jchain(3))
            thunks.append(fin)
            return thunks

        # Alternate ascending/descending chunk order per head: compute can
        # start as soon as the first (h0, c0) input slices land, every big
        # c3-QK start is cushioned by the previous head's big c3-AV (and
        # small c0-QK starts follow tiny c0-AVs), and the kernel tail is the
        # SHORT chunk-0 AV.
        pending = deque()
        for h in range(HL):
            order = range(NCH) if h == 0 else reversed(range(NCH))
            for c in order:
                # (h0, c1): don't interleave chunk-0's AV into the QK groups —
                # those thunks wait on the late-arriving v16 DMA and would
                # stall the PE mid-QK; drained after the groups instead.
                work = deque() if (h, c) == (0, 1) else pending
                pt = emit_qk_exp(h, c, work)
                while pending:
                    pending.popleft()()
                pending = make_av_thunks(h, c, pt)
                if h == 0 and c == 0:
                    late_dmas(0)
                elif h == 0 and c == 1:
                    late_dmas(1)
        while pending:
            pending.popleft()()

    split_multi_waits(nc)
    return nc


def _make_mask() -> np.ndarray:
    kp = np.arange(P)[:, None]
    n = np.arange(P)[None, :]
    return np.where(kp > n, NEG, 0.0).astype(np.float16)


def _make_mask2() -> np.ndarray:
    p = np.arange(P)[:, None]
    u = np.arange(CHUNK)[None, :]
    return np.where(u >= p, B_U8, MASK_NEG).astype(np.float16)


def core_inputs(q, k, v, core):
    h0 = core * HL
    qTh = np.ascontiguousarray(q[:, h0 : h0 + HL, :].transpose(1, 2, 0)).astype(
        np.float16
    )
    kTh = np.ascontiguousarray(k[:, core, :].T).astype(np.float16)
    vxh = np.zeros((S, VW), dtype=np.float32)
    vxh[:, :D] = v[:, core, :]
    vxh[:, D] = 1.0
    return {
        "qT": qTh,
        "kT": kTh,
        "vx8": vxh.astype(NPF8),
        "vx16": vxh[:CHUNK].astype(np.float16),
        "mask": _make_mask(),
        "ident": np.eye(P, dtype=np.float16),
        "mask2": _make_mask2(),
    }


_NC = None


def _get_nc():
    global _NC
    if _NC is None:
        _NC = build_nc()
    return _NC


def make_in_maps(q, k, v):
    return [core_inputs(q, k, v, c) for c in range(N_CORES)]


def run(in_maps, **kwargs):
    return run_bass_kernel_spmd(_get_nc(), in_maps, list(range(N_CORES)), **kwargs)


def kernel(q: np.ndarray, k: np.ndarray, v: np.ndarray) -> np.ndarray:
    q = np.asarray(q, dtype=np.float32)
    k = np.asarray(k, dtype=np.float32)
    v = np.asarray(v, dtype=np.float32)
    res = run(make_in_maps(q, k, v))
    out = np.empty((S, N_CORES * HL * D), dtype=np.float32)
    for core in range(N_CORES):
        oxc = np.asarray(res.results[core]["ox"], dtype=np.float32)  # [HL,S,VW]
        for h in range(HL):
            col = (core * HL + h) * D
            out[:, col : col + D] = oxc[h, :, :D] / oxc[h, :, D : D + 1]
    return out


# revision 52
# speedup vs baseline: 1.0434x; 1.0147x over previous
"""Causal GQA attention (S=2048, Hq=32, Hkv=8, D=128, fp32 IO) on 8 Trainium2
NeuronCores, sharded over heads: core i handles q-heads 4i..4i+3 and kv-head i
(no cross-core communication).

Per-core Bass/Tile kernel design (v2 — fp8 + dual-engine exp):
- Scores are computed TRANSPOSED (S^T[k, q], d on the contraction partitions)
  from host-pre-transposed fp16 Q/K, exactly as the fp16 baseline.
- The exp is split across BOTH the scalar engine (native Exp activation with a
  folded -1 bias, fp8e4 output) and the vector engine (Schraudolph fast-exp:
  one tensor_scalar computing round(score*A + B) into a uint8 tile whose bytes
  ARE the fp8e4 encoding of 2^((u-56)/8) ~= exp(score*SCALE - 1); the fp32->u8
  conversion saturates at 0/255 and rounds-to-nearest, HW-verified). Causal
  triangle masking for DVE-handled diagonal tiles is folded into the same op
  via scalar_tensor_tensor with a precomputed additive mask (-1e9 above the
  diagonal -> u8 0 -> +0.0 in fp8).
- The P*V accumulation runs in fp8 with MatmulPerfMode.DoubleRow: each matmul
  contracts TWO 128-row k-tiles (stationary = paired P^T tiles, moving =
  paired V tiles extended with a ones column that yields the softmax
  denominator for free in column 128).
- Early queries (chunk 0, q < 512) have too few softmax terms to average away
  fp8 noise, so chunk 0 runs the original fp16 path (fp16 pt, fp16 V, scalar
  exp, -30000 identity-matmul triangle masking).
- The AV PSUM quad [128, 4, 512] is copied to SBUF fp16 UNNORMALIZED (with the
  denominator riding in column 128) and DMA'd out; the host does the divide.
"""

from collections import deque
from contextlib import ExitStack

import numpy as np

import concourse.bass as bass
import concourse.mybir as mybir
import concourse.tile as tile
from concourse.mybir import ActivationFunctionType as AF
from concourse.alu_op_type import AluOpType
from concourse.vector_clock import ScopedClock
from concourse.bass_utils import run_bass_kernel_spmd

# Walrus's BIR-simulation pass is ~85% of NEFF compile time and is a
# verification-only pass; skip it.
try:
    import concourse.bass_utils as _bu

    if not getattr(_bu, "_birsim_patched", False):
        _orig_run_command = _bu.run_command

        def _fast_run_command(cmd, *a, **kw):
            cmd = [
                c.replace("--enable-birsim=true", "--enable-birsim=false")
                if isinstance(c, str)
                else c
                for c in cmd
            ]
            return _orig_run_command(cmd, *a, **kw)

        _bu.run_command = _fast_run_command
        _bu._birsim_patched = True
except Exception:
    pass

S = 2048
D = 128
P = 128
NT = S // P          # 16 k-tiles
CHUNK = 512          # q columns per score chunk
NCH = S // CHUNK     # 4 chunks
TPC = CHUNK // P     # 4 k-tiles / diag rows per chunk
VW = 132             # v_ext free width (128 d + 1 ones + 3 pad)
G = 2                # k-tiles per PSUM score group (2 banks; av quad takes 4)
SCALE = 0.08838834764831845
NEG = -30000.0
HL = 4               # q-heads per core
N_CORES = 8

LN2 = 0.6931471805599453
C_SHIFT = 1.0                      # exp(s*SCALE - C): keeps fp8 range safe
A_U8 = 8.0 * SCALE / LN2           # Schraudolph multiplier
B_U8 = 56.0 - 0.52 - 8.0 * C_SHIFT / LN2   # bias incl. -0.52 centering
MASK_NEG = -60000.0                # fp16-representable; a*s+MASK_NEG -> u8 0

F16 = mybir.dt.float16
F32 = mybir.dt.float32
F8 = mybir.dt.float8e4
U8 = mybir.dt.uint8
NPF8 = mybir.dt.np(F8)

WAIT_LIMIT = 1  # this image's walrus encodes at most one sync-wait per inst

# Engine-balance cost model (ns); static greedy assignment. HW-calibrated:
# scalar ACTIVATE = n*0.833 + 293; DVE tensor_scalar/stt = n*1.042 + 190
# (independent of output dtype / bias).
ACT_FIX, ACT_PER = 293.0, 0.833
DVE_FIX, DVE_PER = 190.0, 1.042


class SplitDrainTileContext(tile.TileContext):
    """TileContext whose exit drain spreads its semaphore waits over
    multiple SP instructions (walrus here caps sync-waits per inst)."""

    def _drain_and_barrier(self, tick_clock, wait_clock):
        drain_inst = self.nc.sync.drain()
        wait_clock.add_sem_waits(
            drain_inst.ins, ScopedClock({None: tick_clock.global_clock})
        )
        waits = list(drain_inst.ins.sync_info.on_wait)
        if len(waits) > WAIT_LIMIT:
            drain_inst.ins.sync_info = mybir.SyncInfo(
                on_wait=waits[:WAIT_LIMIT],
                on_update=list(drain_inst.ins.sync_info.on_update),
            )
            for i in range(WAIT_LIMIT, len(waits), WAIT_LIMIT):
                nop = self.nc.sync.nop(nofuse=True)
                nop.ins.sync_info = mybir.SyncInfo(
                    on_wait=waits[i : i + WAIT_LIMIT], on_update=[]
                )
        self.nc.all_engine_barrier()
        popped = self.nc._tile_sem_poison_stack.pop()
        assert popped is self._sem_poison
        self.nc.clear_and_free_semaphores(list(self.sems.allocated().values()))
        self.nc.all_engine_barrier()


def split_multi_waits(nc, limit: int = WAIT_LIMIT):
    """Spread >limit sync-waits onto same-engine NOPs inserted before the
    instruction (engines execute in order: cumulative semantics identical)."""
    n_split = 0
    for fn in nc.m.functions:
        for bb in fn.blocks:
            out = []
            changed = False
            for inst in bb.instructions:
                si = inst.sync_info
                waits = list(si.on_wait) if si is not None else []
                if len(waits) > limit:
                    changed = True
                    n_split += 1
                    extra = waits[:-limit]
                    for ci in range(0, len(extra), limit):
                        nop = mybir.InstNoOp(
                            name=f"{inst.name}-sw{ci}", ins=[], outs=[]
                        )
                        nop.engine = inst.engine
                        nop.sync_info = mybir.SyncInfo(
                            on_wait=extra[ci : ci + limit], on_update=[]
                        )
                        nc.register_instruction(nop, overwrite=True)
                        out.append(nop)
                    inst.sync_info = mybir.SyncInfo(
                        on_wait=waits[-limit:], on_update=list(si.on_update)
                    )
                out.append(inst)
            if changed:
                bb.instructions = out
    return n_split


def build_nc() -> bass.Bass:
    nc = bass.Bass()

    qT = nc.dram_tensor("qT", [HL, P, S], F16, kind="ExternalInput")
    kT = nc.dram_tensor("kT", [P, S], F16, kind="ExternalInput")
    vx8 = nc.dram_tensor("vx8", [S, VW], F8, kind="ExternalInput")
    vx16 = nc.dram_tensor("vx16", [CHUNK, VW], F16, kind="ExternalInput")
    mask = nc.dram_tensor("mask", [P, P], F16, kind="ExternalInput")
    ident = nc.dram_tensor("ident", [P, P], F16, kind="ExternalInput")
    # mask2[p, u] = B_U8 if u >= p else MASK_NEG; diag tile r at window
    # [off:] uses mask2[:, :CHUNK-off] (value depends only on j-off-p).
    mask2 = nc.dram_tensor("mask2", [P, CHUNK], F16, kind="ExternalInput")
    ox = nc.dram_tensor("ox", [HL, S, VW], F16, kind="ExternalOutput")

    # running per-engine cost estimates for static load balancing
    est = {"s": 0.0, "v": 0.0}

    def pick(scalar_cost, dve_cost):
        if est["s"] + scalar_cost <= est["v"] + dve_cost:
            est["s"] += scalar_cost
            return "s"
        est["v"] += dve_cost
        return "v"

    with SplitDrainTileContext(nc) as tc, ExitStack() as ctx:
        const = ctx.enter_context(tc.tile_pool(name="const", bufs=1))
        qpool = ctx.enter_context(tc.tile_pool(name="qpool", bufs=HL))
        pt8pool = ctx.enter_context(tc.tile_pool(name="pt8pool", bufs=2))
        pt16pool = ctx.enter_context(tc.tile_pool(name="pt16pool", bufs=2))
        opool = ctx.enter_context(tc.tile_pool(name="opool", bufs=2))
        psum_sc = ctx.enter_context(tc.tile_pool(name="psc", bufs=3, space="PSUM"))
        psum_av = ctx.enter_context(tc.tile_pool(name="pav", bufs=1, space="PSUM"))

        # Head 0 runs chunks ASCENDING, so the kernel can start computing
        # (h0, c0) after only kT[:, :512] + qT0[:, :512] + the tiny c0-path
        # consts have landed. Input DMAs are split across the sync and gpsimd
        # queues in 512-col pieces ordered by first use.
        kT_sb = const.tile([P, S], F16)
        qT_sbs = []
        qT_sb0 = qpool.tile([P, S], F16, tag="q")
        qT_sbs.append(qT_sb0)
        m_sb = const.tile([P, P], F16)
        i_sb = const.tile([P, P], F16)
        v16_sb = const.tile([P, TPC, VW], F16)
        v8_sb = const.tile([P, NT, VW], F8)
        mask2_sb = const.tile([P, CHUNK], F16)
        nbias = const.tile([P, 1], F32)

        # gpsimd issues NO DMAs (its DGE drain at kernel exit costs ~3.5us
        # once used). Early small pieces ride the scalar queue before its
        # first exp can run; the late bulk (v8, qT0 tail, qT2) is emitted
        # MID-PROGRAM on the scalar queue (see the main loop) where the
        # engine has slack, with est[] charged for the issue time.
        nc.sync.dma_start(kT_sb[:, :CHUNK], kT[:, :CHUNK])
        nc.scalar.dma_start(qT_sb0[:, :CHUNK], qT[0, :, :CHUNK])
        nc.sync.dma_start(m_sb[:], mask[:])
        nc.sync.dma_start(i_sb[:], ident[:])
        nc.vector.memset(nbias[:], -C_SHIFT)
        # prewarm the scalar engine's Exp activation table during the input
        # DMA wait — otherwise the ~1.3us ACT_TABLE_LOAD lands on the
        # critical path right before the first real exp
        warm = const.tile([P, 1], F32)
        nc.scalar.activation(warm[:], nbias[:], AF.Exp, scale=1.0, bias=nbias[:])
        nc.scalar.dma_start(
            qT_sb0[:, CHUNK : 2 * CHUNK], qT[0, :, CHUNK : 2 * CHUNK]
        )
        nc.sync.dma_start(kT_sb[:, CHUNK : 2 * CHUNK], kT[:, CHUNK : 2 * CHUNK])
        nc.scalar.dma_start(mask2_sb[:], mask2[:])
        nc.sync.dma_start(v16_sb[:], vx16.rearrange("(t p) w -> p t w", p=P))
        nc.sync.dma_start(kT_sb[:, 2 * CHUNK :], kT[:, 2 * CHUNK :])
        for h in range(1, HL):
            qT_sb = qpool.tile([P, S], F16, tag="q")
            if h != 2:
                nc.sync.dma_start(qT_sb[:], qT[h])
            qT_sbs.append(qT_sb)

        def late_dmas(step):
            """Mid-program input DMAs on the scalar queue."""
            if step == 0:
                est["s"] += 1100.0
                nc.scalar.dma_start(
                    v8_sb[:], vx8.rearrange("(t p) w -> p t w", p=P)
                )
                est["s"] += 900.0
                nc.scalar.dma_start(
                    qT_sb0[:, 2 * CHUNK :], qT[0, :, 2 * CHUNK :]
                )
            elif step == 1:
                est["s"] += 1700.0
                nc.scalar.dma_start(qT_sbs[2][:], qT[2])

        def emit_qk_exp(h, c, work):
            """QK + exp for one (head, chunk). After each PSUM score group is
            emitted, one pending AV thunk from the previous chunk is emitted
            so the PE interleaves AV matmuls with QK instead of stalling on
            the exp engines draining score PSUM (stalls also drop the PE out
            of its fast p-state).

            c == 0: fp16 path (trimmed QK + identity-matmul triangle mask,
            scalar exp -> fp16 pt). c >= 1: fp8 path (no mask matmuls;
            full-tile groups exp'd on scalar Exp->fp8 or DVE Schraudolph->u8;
            diagonal tiles on DVE scalar_tensor_tensor with the mask4
            additive table)."""
            qT_sb = qT_sbs[h]
            ntiles = TPC * (c + 1)
            if c == 0:
                pt = pt16pool.tile([P, TPC, CHUNK], F16, tag="pt16")
            else:
                pt = pt8pool.tile([P, NT, CHUNK], F8, tag="pt8")
            for t0 in range(0, ntiles, G):
                ng = min(G, ntiles - t0)
                sc = psum_sc.tile([P, G, CHUNK], F32, tag="sc")
                for idx in range(ng):
                    t = t0 + idx
                    r = t - TPC * c
                    if r >= 0:
                        off = P * r
                        nc.tensor.matmul(
                            sc[:, idx, off:],
                            kT_sb[:, t * P : (t + 1) * P],
                            qT_sb[:, c * CHUNK + off : (c + 1) * CHUNK],
                            start=True,
                            stop=not (c == 0),
                        )
                        if c == 0:
                            nc.tensor.matmul(
                                sc[:, idx, off : off + P],
                                i_sb[:],
                                m_sb[:],
                                start=False,
                                stop=True,
                            )
                    else:
                        nc.tensor.matmul(
                            sc[:, idx, :],
                            kT_sb[:, t * P : (t + 1) * P],
                            qT_sb[:, c * CHUNK : (c + 1) * CHUNK],
                            start=True,
                            stop=True,
                        )
                nfull = sum(1 for idx in range(ng) if (t0 + idx) < TPC * c)
                if nfull:
                    n = nfull * CHUNK
                    eng = pick(ACT_FIX + ACT_PER * n, DVE_FIX + DVE_PER * n)
                    if eng == "s":
                        nc.scalar.activation(
                            pt[:, t0 : t0 + nfull, :],
                            sc[:, :nfull, :],
                            AF.Exp,
                            scale=SCALE,
                            bias=nbias[:],
                        )
                    else:
                        nc.vector.tensor_scalar(
                            pt[:, t0 : t0 + nfull, :].bitcast(U8),
                            sc[:, :nfull, :],
                            A_U8,
                            B_U8,
                            AluOpType.mult,
                            AluOpType.add,
                        )
                for idx in range(nfull, ng):
                    t = t0 + idx
                    r = t - TPC * c
                    off = P * r
                    n = CHUNK - off
                    if c == 0:
                        est["s"] += ACT_FIX + ACT_PER * n
                        nc.scalar.activation(
                            pt[:, t, off:],
                            sc[:, idx, off:],
                            AF.Exp,
                            scale=SCALE,
                            bias=nbias[:],
                        )
                    else:
                        est["v"] += DVE_FIX + DVE_PER * n
                        nc.vector.scalar_tensor_tensor(
                            pt[:, t, off:].bitcast(U8),
                            sc[:, idx, off:],
                            A_U8,
                            mask2_sb[:, : CHUNK - off],
                            AluOpType.mult,
                            AluOpType.add,
                        )
                if work:
                    work.popleft()()
            return pt

        def make_av_thunks(h, c, pt):
            """AV for one (head, chunk) as thunks: j-subtile accumulation
            chains into the 2-bank av PSUM pair, with a converting copy after
            each pair of chains and one DMA at the end. Emitted interleaved
            between the NEXT chunk's QK groups."""
            av = psum_av.tile([P, 2, CHUNK], F32, tag="av")
            o_ext = opool.tile([P, TPC, VW], F16, tag="o")
            thunks = deque()

            def jchain(j, half=None):
                """half=0/1 splits the accumulation chain for finer PE
                interleaving (half 0 emits start, half 1 emits stop)."""
                nk = TPC * c + j + 1
                slot = av[:, j % 2, :VW]
                if c == 0:
                    for t in range(nk):
                        nc.tensor.matmul(
                            slot,
                            pt[:, t, j * P : (j + 1) * P],
                            v16_sb[:, t, :],
                            start=(t == 0),
                            stop=(t == nk - 1),
                        )
                    return
                npair = nk // 2
                odd = nk % 2
                mid = (npair + 1) // 2
                rng = (
                    range(npair)
                    if half is None
                    else (range(mid) if half == 0 else range(mid, npair))
                )
                for m in rng:
                    nc.tensor.matmul(
                        slot,
                        pt[:, 2 * m : 2 * m + 2, j * P : (j + 1) * P],
                        v8_sb[:, 2 * m : 2 * m + 2, :],
                        start=(m == 0),
                        stop=(m == npair - 1 and not odd),
                        perf_mode=mybir.MatmulPerfMode.DoubleRow,
                    )
                if odd and half != 0:
                    nc.tensor.matmul(
                        slot,
                        pt[:, nk - 1, j * P : (j + 1) * P],
                        v8_sb[:, nk - 1, :],
                        start=(nk == 1),
                        stop=True,
                    )

            def copy_pair(jp):
                n = 2 * VW
                eng = pick(ACT_FIX + ACT_PER * n, DVE_FIX + DVE_PER * n)
                if eng == "s":
                    nc.scalar.copy(o_ext[:, 2 * jp : 2 * jp + 2, :], av[:, :, :VW])
                else:
                    nc.vector.tensor_scalar_mul(
                        o_ext[:, 2 * jp : 2 * jp + 2, :], av[:, :, :VW], 1.0
                    )

            def fin():
                copy_pair(1)
                nc.sync.dma_start(
                    ox[h, c * CHUNK : (c + 1) * CHUNK, :].rearrange(
                        "(j p) w -> p j w", p=P
                    ),
                    o_ext[:],
                )

            thunks.append(lambda: jchain(0))
            thunks.append(lambda: jchain(1))
            thunks.append(lambda: copy_pair(0))
            thunks.append(lambda: jchain(2))
            thunks.append(lambda: jchain(3))
            thunks.append(fin)
            return thunks

        # Alternate ascending/descending chunk order per head: compute can
        # start as soon as the first (h0, c0) input slices land, every big
        # c3-QK start is cushioned by the previous head's big c3-AV (and
        # small c0-QK starts follow tiny c0-AVs), and the kernel tail is the
        # SHORT chunk-0 AV.
        pending = deque()
        for h in range(HL):
            order = range(NCH) if h == 0 else reversed(range(NCH))
            for c in order:
                # (h0, c1): don't interleave chunk-0's AV into the QK groups —
                # those thunks wait on the late-arriving v16 DMA and would
                # stall the PE mid-QK; drained after the groups instead.
                work = deque() if (h, c) == (0, 1) else pending
                pt = emit_qk_exp(h, c, work)
                while pending:
                    pending.popleft()()
                pending = make_av_thunks(h, c, pt)
                if h == 0 and c == 0:
                    late_dmas(0)
                elif h == 0 and c == 1:
                    late_dmas(1)
        while pending:
            pending.popleft()()

    split_multi_waits(nc)
    return nc


def _make_mask() -> np.ndarray:
    kp = np.arange(P)[:, None]
    n = np.arange(P)[None, :]
    return np.where(kp > n, NEG, 0.0).astype(np.float16)


def _make_mask2() -> np.ndarray:
    p = np.arange(P)[:, None]
    u = np.arange(CHUNK)[None, :]
    return np.where(u >= p, B_U8, MASK_NEG).astype(np.float16)


def core_inputs(q, k, v, core):
    h0 = core * HL
    qTh = np.ascontiguousarray(q[:, h0 : h0 + HL, :].transpose(1, 2, 0)).astype(
        np.float16
    )
    kTh = np.ascontiguousarray(k[:, core, :].T).astype(np.float16)
    vxh = np.zeros((S, VW), dtype=np.float32)
    vxh[:, :D] = v[:, core, :]
    vxh[:, D] = 1.0
    return {
        "qT": qTh,
        "kT": kTh,
        "vx8": vxh.astype(NPF8),
        "vx16": vxh[:CHUNK].astype(np.float16),
        "mask": _make_mask(),
        "ident": np.eye(P, dtype=np.float16),
        "mask2": _make_mask2(),
    }


_NC = None


def _get_nc():
    global _NC
    if _NC is None:
        _NC = build_nc()
    return _NC


def make_in_maps(q, k, v):
    return [core_inputs(q, k, v, c) for c in range(N_CORES)]


def run(in_maps, **kwargs):
    return run_bass_kernel_spmd(_get_nc(), in_maps, list(range(N_CORES)), **kwargs)


def kernel(q: np.ndarray, k: np.ndarray, v: np.ndarray) -> np.ndarray:
    q = np.asarray(q, dtype=np.float32)
    k = np.asarray(k, dtype=np.float32)
    v = np.asarray(v, dtype=np.float32)
    res = run(make_in_maps(q, k, v))
    out = np.empty((S, N_CORES * HL * D), dtype=np.float32)
    for core in range(N_CORES):
        oxc = np.asarray(res.results[core]["ox"], dtype=np.float32)  # [HL,S,VW]
        for h in range(HL):
            col = (core * HL + h) * D
            out[:, col : col + D] = oxc[h, :, :D] / oxc[h, :, D : D + 1]
    return out
